# revision 1
# baseline (speedup 1.0000x reference)
"""NT-Xent loss on 8 Trainium2 NeuronCores.

Math (reference): xn = row-normalized x; mat = exp(xn @ xn.T / 0.1) with zero
diagonal; numer_r = mat[r, r±B]; denom_r = column sum r; loss = -mean(log(numer/denom)).

Because mat is symmetric, column sums equal row sums, so a core that owns a
row block [1024, 8192] computes its denominators entirely locally — no
collectives.  Each core c receives x rolled by -1024*c rows so that, in its
local column coordinates, the diagonal sits at col j'=i and the positive pair
at col j'=4096+i for local row i: the special tiles are at the same
compile-time position on every core, keeping the program SPMD-uniform.

Per-core pipeline:
  1. Stream x row-tiles [128,512]; ACT Square+accum row-sum; DVE
     reciprocal + ACT sqrt -> 1/norm; DVE per-partition scale -> xn (bf16).
  2. Transpose xn via PE matmul against identity (out = xn_tile.T @ I),
     PSUM->SBUF copy on DVE -> xnT tiles ([128(d),*] bf16).  Columns 0..1023
     of xnT double as the stationary (lhsT) operand.
  3. For each 1024-wide column pair: matmul accumulate over d (4x128) into
     PSUM [128,1024]; one ACT Exp(scale=10) pass PSUM->SBUF with accum_out
     giving the row-sum; on pair 0 / pair 4 extract the diagonal / positive
     values with a fused DVE multiply-by-identity reduce.
  4. denom = rowsum - diag.  Host applies log and the final mean.
"""

import functools

import ml_dtypes
import numpy as np

N, D, B = 8192, 512, 4096
NCORES = 8
RPC = N // NCORES           # 1024 local rows per core
MB = RPC // 128             # 8 row blocks of 128
NT = N // 128               # 64 row tiles of x
KT = D // 128               # 4 contraction subtiles
PAIRS = N // 1024           # 8 column pairs
NUMER_PAIR = B // 1024      # positive pair lands in column pair 4
TEMP_INV = 10.0             # 1 / temperature


def _build():
    from contextlib import ExitStack

    import concourse.bacc as bacc
    import concourse.mybir as mybir
    import concourse.tile as tile

    F32 = mybir.dt.float32
    BF16 = mybir.dt.bfloat16
    I32 = mybir.dt.int32
    ALU = mybir.AluOpType
    ACTF = mybir.ActivationFunctionType
    AX = mybir.AxisListType

    nc = bacc.Bacc("TRN2", target_bir_lowering=False, debug=False,
                   num_devices=NCORES)
    x_in = nc.dram_tensor("x", [N, D], F32, kind="ExternalInput").ap()
    eye16_in = nc.dram_tensor("eye16", [128, 128], BF16, kind="ExternalInput").ap()
    eye32_in = nc.dram_tensor("eye32", [128, 128], F32, kind="ExternalInput").ap()
    numer_out = nc.dram_tensor("numer", [128, MB], F32, kind="ExternalOutput").ap()
    denom_out = nc.dram_tensor("denom", [128, MB], F32, kind="ExternalOutput").ap()

    with ExitStack() as ctx:
        tc = ctx.enter_context(tile.TileContext(nc))
        consts = ctx.enter_context(tc.tile_pool(name="consts", bufs=1))
        dtp = ctx.enter_context(tc.tile_pool(name="dtp", bufs=1))
        junkp = ctx.enter_context(tc.tile_pool(name="junk", bufs=2))
        xnp = ctx.enter_context(tc.tile_pool(name="xn", bufs=1))
        stats = ctx.enter_context(tc.tile_pool(name="stats", bufs=1))
        lhsp = ctx.enter_context(tc.tile_pool(name="lhs", bufs=1))
        rhsp = ctx.enter_context(tc.tile_pool(name="rhs", bufs=2))
        expp = ctx.enter_context(tc.tile_pool(name="expo", bufs=4))
        pst = ctx.enter_context(tc.tile_pool(name="pst", bufs=3, space="PSUM"))
        psm = ctx.enter_context(tc.tile_pool(name="psm", bufs=2, space="PSUM"))

        eye16 = consts.tile([128, 128], BF16, tag="eye16")
        nc.sync.dma_start(eye16[:], eye16_in)
        eye32 = consts.tile([128, 128], F32, tag="eye32")
        nc.sync.dma_start(eye32[:], eye32_in)

        ss = stats.tile([128, NT], F32, tag="ss")
        invn = stats.tile([128, NT], F32, tag="invn")
        rs = stats.tile([128, MB * PAIRS], F32, tag="rs")
        diagv = stats.tile([128, MB], F32, tag="diagv")
        numv = stats.tile([128, MB], F32, tag="numv")
        rowsum = stats.tile([128, MB], F32, tag="rowsum")
        dent = stats.tile([128, MB], F32, tag="dent")

        xn = [xnp.tile([128, D], BF16, tag=f"xn{i}", name=f"xn{i}")
              for i in range(NT)]
        dts = [dtp.tile([128, 128], BF16, tag=f"dt{i}", name=f"dt{i}")
               for i in range(NT)]

        # Newton-rsqrt scratch (int bit-trick seed, 3 iterations)
        iu = stats.tile([128, NT], I32, tag="iu")
        iv = stats.tile([128, NT], I32, tag="iv")
        nt_t = stats.tile([128, NT], F32, tag="nt_t")

        # Normalize-group emitter: bf16 cast-loads, row sum-of-squares,
        # Newton rsqrt (bit-trick seed), diag(inv) tiles.
        def emit_group(gstart, gsz):
            assert gsz % 4 == 0
            for q in range(gstart // 4, (gstart + gsz) // 4):
                jb = junkp.tile([128, 4 * D], BF16, tag="sqj", name="jb")
                for j in range(4):
                    i = 4 * q + j
                    nc.gpsimd.dma_start(xn[i][:], x_in[i * 128:(i + 1) * 128, :])
                    nc.vector.tensor_mul(jb[:, j * D:(j + 1) * D],
                                         xn[i][:], xn[i][:])
                nc.vector.tensor_reduce(
                    ss[:, 4 * q:4 * q + 4],
                    jb[:].rearrange("p (a b) -> p a b", a=4),
                    axis=AX.X, op=ALU.add)
            sl = slice(gstart, gstart + gsz)
            nc.vector.tensor_scalar(iu[:, sl], ss[:, sl].bitcast(I32), 1, None,
                                    op0=ALU.arith_shift_right)
            nc.vector.tensor_scalar(iv[:, sl], iu[:, sl], -1, 0x5F3759DF,
                                    op0=ALU.mult, op1=ALU.add)
            y = iv[:, sl].bitcast(F32)
            for it in range(3):
                nc.vector.tensor_mul(nt_t[:, sl], y, y)
                nc.vector.tensor_mul(nt_t[:, sl], nt_t[:, sl], ss[:, sl])
                nc.vector.tensor_scalar(nt_t[:, sl], nt_t[:, sl], -0.5, 1.5,
                                        op0=ALU.mult, op1=ALU.add)
                out_y = invn[:, sl] if it == 2 else y
                nc.vector.tensor_mul(out_y, y, nt_t[:, sl])
            for i in range(gstart, gstart + gsz):
                nc.vector.tensor_scalar_mul(dts[i][:], eye16[:],
                                            invn[:, i:i + 1])

        # (gstart, gsz) groups; pair tp consumes tiles [8tp, 8tp+8)
        groups = [(0, 4), (4, 4)] + [(8 * g, 8) for g in range(1, 8)]
        emit_group(*groups[0])
        emit_group(*groups[1])

        lhs = [lhsp.tile([128, RPC], BF16, tag=f"lhs{k}", name=f"lhs{k}")
               for k in range(KT)]

        # Phases 2+3 interleaved per column pair; emit the normalize group
        # feeding pair tp+2 right before pair tp so the DVE queue interleaves.
        for tp in range(PAIRS):
            if tp + 2 < len(groups):
                emit_group(*groups[tp + 2])
            rhs = lhs if tp == 0 else [
                rhsp.tile([128, 1024], BF16, tag=f"rhs{k}", name=f"rhs{k}")
                for k in range(KT)]
            for h in range(2):
                t = 2 * tp + h
                for k in range(KT):
                    ps = pst.tile([128, 512], F32, tag="pst")
                    for j in range(4):
                        nc.tensor.matmul(
                            ps[:, j * 128:(j + 1) * 128],
                            lhsT=xn[4 * t + j][:, k * 128:(k + 1) * 128],
                            rhs=dts[4 * t + j][:], start=True, stop=True)
                    dst = rhs[k][:, h * 512:(h + 1) * 512]
                    if h == 0:
                        nc.scalar.copy(dst, ps[:])
                    else:
                        nc.vector.tensor_copy(dst, ps[:])
            for m in range(MB):
                ps = psm.tile([128, 1024], F32, tag="psm")
                for k in range(KT):
                    for h in range(2):
                        nc.tensor.matmul(
                            ps[:, h * 512:(h + 1) * 512],
                            lhsT=lhs[k][:, m * 128:(m + 1) * 128],
                            rhs=rhs[k][:, h * 512:(h + 1) * 512],
                            start=(k == 0), stop=(k == KT - 1))
                eo = expp.tile([128, 1024], F32, tag="eo")
                col = m * PAIRS + tp
                nc.scalar.activation(eo[:], ps[:], ACTF.Exp, scale=TEMP_INV,
                                     accum_out=rs[:, col:col + 1])
                if tp == 0 or tp == NUMER_PAIR:
                    tgt = diagv if tp == 0 else numv
                    junk = junkp.tile([128, 128], F32, tag="ttj")
                    nc.vector.tensor_mul(junk[:], eo[:, m * 128:(m + 1) * 128],
                                         eye32[:])
                    nc.vector.tensor_reduce(tgt[:, m:m + 1], junk[:],
                                            axis=AX.X, op=ALU.add)

        # Finalize: denom = full row sum - diagonal term.
        for m in range(MB):
            nc.vector.tensor_reduce(rowsum[:, m:m + 1],
                                    rs[:, m * PAIRS:(m + 1) * PAIRS],
                                    axis=AX.X, op=ALU.add)
        nc.vector.tensor_sub(dent[:], rowsum[:], diagv[:])
        nc.sync.dma_start(numer_out, numv[:])
        nc.sync.dma_start(denom_out, dent[:])

    nc.finalize()
    return nc


@functools.lru_cache(maxsize=1)
def _get_nc():
    return _build()


def _run(x, **run_kwargs):
    from concourse.bass_utils import run_bass_kernel_spmd

    x = np.ascontiguousarray(np.asarray(x), dtype=np.float32)
    assert x.shape == (N, D)
    eye16 = np.eye(128, dtype=ml_dtypes.bfloat16)
    eye32 = np.eye(128, dtype=np.float32)
    in_maps = [
        {"x": np.ascontiguousarray(np.roll(x, -c * RPC, axis=0)),
         "eye16": eye16, "eye32": eye32}
        for c in range(NCORES)
    ]
    nc = _get_nc()
    return run_bass_kernel_spmd(nc, in_maps, list(range(NCORES)), **run_kwargs)


def _loss_from_results(results):
    num = np.concatenate(
        [results[c]["numer"].T.reshape(-1) for c in range(NCORES)])
    den = np.concatenate(
        [results[c]["denom"].T.reshape(-1) for c in range(NCORES)])
    loss = -np.sum(np.log(num.astype(np.float64) / den.astype(np.float64))) / N
    return np.float32(loss)


def kernel(x):
    res = _run(x)
    return _loss_from_results(res.results)

